# revision 22
# baseline (speedup 1.0000x reference)
"""Multi-head attention (B=2, S=2048, D=1024, H=16) on 8 TRN2 NeuronCores.

Sharding: DP over batch (2) x TP over heads (4 heads/core).
Core c: batch = c // 4, head group g = c % 4 -> heads [4g, 4g+4).

Per-core kernel (Tile):
  - activations arrive host-pre-transposed as [D, tokens] fp16 so the
    projection matmuls contract over the partition dim directly.
  - Q^T/K^T produced head-pair-stacked [128, 2048] fp16; V produced in
    natural [k, d] layout per head, augmented with a ones column (row 64
    of the PV output is then the softmax denominator for free).
  - S^T tiles [128 k, 2 x 512 q] in PSUM (2 heads row-packed on the PE),
    one ACT pass exp(S/8) -> fp16 (no max subtraction: logits ~ N(0,1)).
  - PV: O^T_h[65, q] accumulated over 16 k-blocks; evacuated with one DVE
    copy (frees PSUM fast), then normalized off the critical path via DVE
    reciprocal + GPSIMD partition-broadcast + DVE multiply.
  - o_proj: per-head 64-contraction matmuls accumulated in PSUM; partial
    [2048, 1024] outputs are summed on host across the 4 TP cores.
  - All projection / o_proj matmul groups are interleaved into the
    attention kb-loops as PE gap fillers so the PE never idles long
    enough for the HAM clock gate to re-throttle.
"""

import sys

if "/opt/trn_rl_repo" not in sys.path:
    sys.path.insert(0, "/opt/trn_rl_repo")

from contextlib import ExitStack

import numpy as np

import concourse.bacc as bacc
import concourse.bass as bass
import concourse.mybir as mybir
import concourse.tile as tile
from concourse.bass_utils import run_bass_kernel_spmd

F32 = mybir.dt.float32
F16 = mybir.dt.float16
EXP = mybir.ActivationFunctionType.Exp
ADD = mybir.AluOpType.add
MULT = mybir.AluOpType.mult

D = 1024  # model dim
TOK = 2048  # tokens per core (one batch element)
HL = 4  # heads per core
DH = 64  # head dim
TC = 512  # token/q chunk
NT = TOK // TC  # 4 q chunks
KB = TOK // 128  # 16 k blocks
NC8 = D // 128  # 8 contraction chunks
N_CORES = 8


def _build():
    nc = bacc.Bacc("TRN2", target_bir_lowering=False, debug=False, num_devices=N_CORES)

    qT_d = nc.declare_dram_parameter("qT", [D, TOK], F16, isOutput=False)
    kT_d = nc.declare_dram_parameter("kT", [D, TOK], F16, isOutput=False)
    vT_d = nc.declare_dram_parameter("vT", [D, TOK], F16, isOutput=False)
    wq_d = nc.declare_dram_parameter("wq", [D, 256], F16, isOutput=False)
    wk_d = nc.declare_dram_parameter("wk", [D, 256], F16, isOutput=False)
    wv_d = nc.declare_dram_parameter("wv", [D, 256], F16, isOutput=False)
    wo_d = nc.declare_dram_parameter("wo", [256, D], F16, isOutput=False)
    bq_d = nc.declare_dram_parameter("bq", [256, 1], F32, isOutput=False)
    bk_d = nc.declare_dram_parameter("bk", [256, 1], F32, isOutput=False)
    bv_d = nc.declare_dram_parameter("bv", [128, 256], F32, isOutput=False)
    bo_d = nc.declare_dram_parameter("bo", [128, D], F32, isOutput=False)
    out_d = nc.declare_dram_parameter("out", [TOK, D], F32, isOutput=True)

    with tile.TileContext(nc) as tc, ExitStack() as ctx:
        wts = ctx.enter_context(tc.tile_pool(name="wts", bufs=3))
        wop = ctx.enter_context(tc.tile_pool(name="wop", bufs=2))
        bp = ctx.enter_context(tc.tile_pool(name="bp", bufs=4))
        bvp = ctx.enter_context(tc.tile_pool(name="bvp", bufs=1))
        bop = ctx.enter_context(tc.tile_pool(name="bop", bufs=1))
        actp = ctx.enter_context(tc.tile_pool(name="actp", bufs=6))
        qkt = ctx.enter_context(tc.tile_pool(name="qkt", bufs=4))
        vga = ctx.enter_context(tc.tile_pool(name="vga", bufs=KB))
        p4p = ctx.enter_context(tc.tile_pool(name="p4p", bufs=6))
        otp = ctx.enter_context(tc.tile_pool(name="otp", bufs=4))
        oup = ctx.enter_context(tc.tile_pool(name="oup", bufs=2))
        rcp = ctx.enter_context(tc.tile_pool(name="rcp", bufs=2))
        bcp = ctx.enter_context(tc.tile_pool(name="bcp", bufs=2))
        ozp = ctx.enter_context(tc.tile_pool(name="ozp", bufs=4))
        pp = ctx.enter_context(tc.tile_pool(name="pp", bufs=2, space="PSUM"))
        ps = ctx.enter_context(tc.tile_pool(name="ps", bufs=2, space="PSUM"))
        po = ctx.enter_context(tc.tile_pool(name="po", bufs=1, space="PSUM"))

        def load_w(dram):
            # one 3D-AP DMA for all 8 contraction chunks: tile free dim is
            # (c8, col), DRAM partition dim is c8*128 + p.
            t_ = wts.tile([128, NC8 * 256], F16, tag="wts", name="wts_t")
            nc.sync.dma_start(
                t_[:].rearrange("p (c n) -> p c n", n=256),
                dram[:, :].rearrange("(c p) n -> p c n", p=128),
            )
            return [t_[:, c8 * 256 : (c8 + 1) * 256] for c8 in range(NC8)]

        def load_b(dram):
            lst = []
            for pair in range(2):
                t_ = bp.tile([128, 1], F32, tag="bp", name="b_t")
                nc.sync.dma_start(t_[:], dram[pair * 128 : (pair + 1) * 128, :])
                lst.append(t_)
            return lst

        KT = [qkt.tile([128, TOK], F16, tag="qkt", name=f"KT{i}") for i in range(2)]
        QT = [qkt.tile([128, TOK], F16, tag="qkt", name=f"QT{i}") for i in range(2)]

        def act_dma_set(src_d, t):
            ch = actp.tile([128, NC8 * TC], F16, tag="actp", name="act_ch")
            nc.sync.dma_start(
                ch[:].rearrange("p (c n) -> p c n", n=TC),
                src_d[:, t * TC : (t + 1) * TC].rearrange("(c p) n -> p c n", p=128),
            )
            return [ch[:, c8 * TC : (c8 + 1) * TC] for c8 in range(NC8)]

        def qk_half(dst, w_sb, b_sb, chunks, t, pair, half, box):
            if half == 0:
                box[1] = pp.tile([128, TC], F32, tag="pp", name="proj_ps")
            pt = box[1]
            for c8 in range(4 * half, 4 * half + 4):
                nc.tensor.matmul(
                    pt[:],
                    w_sb[c8][:, pair * 128 : (pair + 1) * 128],
                    chunks[c8],
                    start=(c8 == 0),
                    stop=(c8 == NC8 - 1),
                )
            if half == 1:
                nc.vector.tensor_scalar_add(
                    dst[pair][:, t * TC : (t + 1) * TC], pt[:], b_sb[pair][:]
                )

        def qk_pair(dst, w_sb, b_sb, chunks, t, pair):
            box = [None, None]
            qk_half(dst, w_sb, b_sb, chunks, t, pair, 0, box)
            qk_half(dst, w_sb, b_sb, chunks, t, pair, 1, box)

        # --- head of pipeline: K(0), Q(0), V chunks 0-1. Weight and
        # first-chunk DMAs are interleaved so the first matmul can start
        # after two transfers instead of eighteen. ---------------------
        # PE warm-up: junk matmuls during the DMA lead-in keep the HAM
        # activity monitor at full clock so real matmuls start warm.
        wu_in = wts.tile([128, TC], F16, tag="wu", name="wu_in")
        nc.vector.memset(wu_in[:], 0.0)
        wu_ps = pp.tile([128, TC], F32, tag="pp", name="wu_ps")
        for i in range(56):
            nc.tensor.matmul(
                wu_ps[:], wu_in[:, 0:128], wu_in[:], start=(i == 0), stop=(i == 55)
            )

        wk_sb = load_w(wk_d)
        kchunks0 = act_dma_set(kT_d, 0)
        bk_sb = load_b(bk_d)
        qk_pair(KT, wk_sb, bk_sb, kchunks0, 0, 0)
        qk_pair(KT, wk_sb, bk_sb, kchunks0, 0, 1)
        wq_sb = load_w(wq_d)
        qchunks0 = act_dma_set(qT_d, 0)
        bq_sb = load_b(bq_d)
        qk_pair(QT, wq_sb, bq_sb, qchunks0, 0, 0)
        qk_pair(QT, wq_sb, bq_sb, qchunks0, 0, 1)

        wv_sb = load_w(wv_d)
        bv_sb = bvp.tile([128, 256], F32, tag="bvp", name="bv_sb")
        nc.sync.dma_start(bv_sb[:], bv_d[:, :])
        wo_sb = []  # per head pair: [128, 1024]
        for hp in range(2):
            t_ = wop.tile([128, D], F16, tag="wop", name="wo_t")
            nc.sync.dma_start(t_[:], wo_d[hp * 128 : (hp + 1) * 128, :])
            wo_sb.append(t_)
        bo_sb = bop.tile([128, D], F32, tag="bop", name="bo_sb")
        nc.sync.dma_start(bo_sb[:], bo_d[:, :])

        vag = {}
        vchunk_sets = {}

        def v_dma_set(vt):
            vchunk_sets[vt] = act_dma_set(vT_d, vt)

        def v_group(kb):
            vt, j = divmod(kb, 4)
            vchunks = vchunk_sets[vt]
            pv = pp.tile([128, 256], F32, tag="pp", name="vproj_ps")
            for c8 in range(NC8):
                nc.tensor.matmul(
                    pv[:],
                    vchunks[c8][:, j * 128 : (j + 1) * 128],
                    wv_sb[c8],
                    start=(c8 == 0),
                    stop=(c8 == NC8 - 1),
                )
            # all 4 heads' V slices + bias in one strided DVE op; the ones
            # columns (one per head, stride 65) are set separately.
            vt_ = vga.tile([128, 4 * 65], F16, tag="vga", name="vag_t")
            dst = vt_[:].rearrange("p (h c) -> p h c", c=65)[:, :, 0:64]
            nc.vector.tensor_tensor(
                dst,
                pv[:].rearrange("p (h c) -> p h c", c=64),
                bv_sb[:].rearrange("p (h c) -> p h c", c=64),
                op=ADD,
            )
            nc.vector.memset(vt_[:].rearrange("p (h c) -> p h c", c=65)[:, :, 64:65], 1.0)
            for h in range(HL):
                vag[(h, kb)] = vt_[:, h * 65 : h * 65 + 65]

        v_dma_set(0)
        v_dma_set(1)
        v_dma_set(2)
        for kb in range(8):
            v_group(kb)

        ot_store = {}

        def oproj_group(t, qbl, f2):
            qb = 4 * t + qbl
            pz = pp.tile([128, TC], F32, tag="pp", name="oproj_ps")
            for hp in range(2):
                nc.tensor.matmul(
                    pz[:],
                    ot_store[(t, hp)][:, qbl * 128 : (qbl + 1) * 128],
                    wo_sb[hp][:, f2 * TC : (f2 + 1) * TC],
                    start=(hp == 0),
                    stop=(hp == 1),
                )
            oz = ozp.tile([128, TC], F32, tag="ozp", name="oz")
            nc.vector.tensor_tensor(
                oz[:], pz[:], bo_sb[:, f2 * TC : (f2 + 1) * TC], op=ADD
            )
            eng = nc.gpsimd if (qbl + f2) % 2 == 0 else nc.sync
            eng.dma_start(
                out_d[qb * 128 : (qb + 1) * 128, f2 * TC : (f2 + 1) * TC], oz[:]
            )

        def attn_pass(t, hp, fillers):
            """One head-pair kb-loop; fillers[kb] emits extra PE work.

            The S^T matmuls run two kb ahead of the PV matmuls (matching
            the two s4 PSUM slots) so a PV blocked on the accumulator WAR
            at pass start does not stall S^T/exp behind it on the
            in-order engines."""
            pO = po.tile([65, 1024], F32, tag="po", name="pO")
            p4s = {}

            def s_exp(kb):
                s4 = ps.tile([128, 1024], F32, tag="ps", name="s4")
                for par in range(2):
                    rows = slice(par * 64, par * 64 + 64)
                    nc.tensor.matmul(
                        s4[:, par * TC : (par + 1) * TC],
                        KT[hp][rows, kb * 128 : (kb + 1) * 128],
                        QT[hp][rows, t * TC : (t + 1) * TC],
                        start=True,
                        stop=True,
                    )
                p4 = p4p.tile([128, 1024], F16, tag="p4p", name="p4")
                nc.scalar.activation(p4[:], s4[:], EXP, scale=0.125)
                p4s[kb] = p4

            s_exp(0)
            for kb in range(KB):
                for f in fillers.get(kb, ()):
                    f()
                if kb + 1 < KB:
                    s_exp(kb + 1)
                p4 = p4s.pop(kb)
                for par in range(2):
                    h = 2 * hp + par
                    nc.tensor.matmul(
                        pO[:, par * TC : (par + 1) * TC],
                        vag[(h, kb)],
                        p4[:, par * TC : (par + 1) * TC],
                        start=(kb == 0),
                        stop=(kb == KB - 1),
                    )
            # fast evacuation frees the PSUM accumulator quickly; the
            # normalization chain runs off the PE critical path. The denom
            # row moves to partition 0 (partition_broadcast requires it).
            # On the very last pass nothing waits on the PSUM slot, so the
            # mults read PSUM directly and the copy is skipped.
            last = t == NT - 1 and hp == 1
            if not last:
                ou = oup.tile([64, 1024], F32, tag="oup", name="ou")
                nc.vector.tensor_copy(ou[:], pO[0:64, :])
            else:
                ou = pO
            rc = rcp.tile([1, 1024], F32, tag="rcp", name="rc")
            nc.vector.tensor_copy(rc[0:1, :], pO[64:65, :])
            bd = bcp.tile([64, 1024], F32, tag="bcp", name="bd")
            nc.gpsimd.partition_broadcast(bd[:], rc[:], channels=64)
            bc = bcp.tile([64, 1024], F32, tag="bcp", name="bc")
            nc.vector.reciprocal_approx_fast(bc[:], bd[:])
            # pair-stacked normalized output: even head -> partitions 0:64,
            # odd head -> 64:128 (DVE partition-shifted write), so o_proj
            # contracts 128 rows per matmul.
            ot = otp.tile([128, TC], F16, tag="otp", name="ot")
            for par in range(2):
                nc.vector.tensor_tensor(
                    ot[par * 64 : (par + 1) * 64, :],
                    ou[0:64, par * TC : (par + 1) * TC],
                    bc[:, par * TC : (par + 1) * TC],
                    op=MULT,
                )
            ot_store[(t, hp)] = ot

        def kq_filler(src_d, dst, w_sb, b_sb, t, pair, half, box):
            def f():
                if box[0] is None:
                    box[0] = act_dma_set(src_d, t)
                qk_half(dst, w_sb, b_sb, box[0], t, pair, half, box)

            return f

        kbox = {t: [None, None] for t in (1, 2, 3)}
        qbox = {t: [None, None] for t in (1, 2, 3)}

        def kf(t, pair, half):
            return kq_filler(kT_d, KT, wk_sb, bk_sb, t, pair, half, kbox[t])

        def qf(t, pair, half):
            return kq_filler(qT_d, QT, wq_sb, bq_sb, t, pair, half, qbox[t])

        f00 = {
            0: [kf(1, 0, 0)],
            1: [kf(1, 0, 1)],
            2: [kf(1, 1, 0), lambda: v_dma_set(3)],
            3: [kf(1, 1, 1), lambda: v_group(8)],
            4: [kf(2, 0, 0), lambda: v_group(9)],
            5: [kf(2, 0, 1)],
            6: [kf(2, 1, 0), lambda: v_group(10)],
            7: [kf(2, 1, 1), lambda: v_group(11)],
            8: [lambda: v_group(12)],
            9: [lambda: v_group(13)],
            10: [kf(3, 0, 0), lambda: v_group(14)],
            11: [kf(3, 0, 1), lambda: v_group(15)],
            12: [kf(3, 1, 0)],
            13: [kf(3, 1, 1)],
        }
        # o_proj(t-1) depends on ot tiles that finish ~6us after pass
        # (t-1, hp1) ends (copy->bcast->recip->mult), so its groups go in
        # late hp0 slots / early hp1 slots of the next t. Q(t+1) projection
        # halves are spread across both passes for even PE load.
        for t in range(NT):
            if t == 0:
                f_hp0 = f00
                f_hp1 = {2 * j: [qf(1, j // 2, j % 2)] for j in range(4)}
            else:
                f_hp0 = {
                    7 + 2 * j: [lambda i=j, t=t: oproj_group(t - 1, i // 2, i % 2)]
                    for j in range(4)
                }
                f_hp1 = {}
                if t + 1 < NT:
                    for j in range(4):
                        f_hp0.setdefault(2 * j, []).append(qf(t + 1, j // 2, j % 2))
                for j in range(4):
                    f_hp1.setdefault(2 * j + 1, []).append(
                        lambda i=4 + j, t=t: oproj_group(t - 1, i // 2, i % 2)
                    )
            attn_pass(t, 0, f_hp0)
            attn_pass(t, 1, f_hp1)
        for i in range(8):
            oproj_group(NT - 1, i // 2, i % 2)

    nc.compile()
    return nc


_NC = None


def _get_nc():
    global _NC
    if _NC is None:
        _NC = _build()
    return _NC


def _shard(inputs):
    q = np.asarray(inputs["q"], np.float32)
    k = np.asarray(inputs["k"], np.float32)
    v = np.asarray(inputs["v"], np.float32)
    Wq = np.asarray(inputs["Wq"], np.float32)
    Wk = np.asarray(inputs["Wk"], np.float32)
    Wv = np.asarray(inputs["Wv"], np.float32)
    Wo = np.asarray(inputs["Wo"], np.float32)
    bq = np.asarray(inputs["bq"], np.float32)
    bk = np.asarray(inputs["bk"], np.float32)
    bv = np.asarray(inputs["bv"], np.float32)
    bo = np.asarray(inputs["bo"], np.float32)

    qT = [np.ascontiguousarray(q[b].T).astype(np.float16) for b in range(2)]
    kT = [np.ascontiguousarray(k[b].T).astype(np.float16) for b in range(2)]
    vT = [np.ascontiguousarray(v[b].T).astype(np.float16) for b in range(2)]
    bo_bc = np.tile((bo / 4.0).reshape(1, D), (128, 1)).astype(np.float32)

    in_maps = []
    for c in range(N_CORES):
        b, g = divmod(c, 4)
        sl = slice(g * 256, (g + 1) * 256)
        in_maps.append(
            {
                "qT": qT[b],
                "kT": kT[b],
                "vT": vT[b],
                "wq": np.ascontiguousarray(Wq[:, sl]).astype(np.float16),
                "wk": np.ascontiguousarray(Wk[:, sl]).astype(np.float16),
                "wv": np.ascontiguousarray(Wv[:, sl]).astype(np.float16),
                "wo": np.ascontiguousarray(Wo[sl, :]).astype(np.float16),
                "bq": np.ascontiguousarray(bq[sl].reshape(256, 1)),
                "bk": np.ascontiguousarray(bk[sl].reshape(256, 1)),
                "bv": np.tile(bv[sl].reshape(1, 256), (128, 1)).astype(np.float32),
                "bo": bo_bc,
            }
        )
    return in_maps


def _run(inputs, trace=False, **kwargs):
    nc = _get_nc()
    in_maps = _shard(inputs)
    res = run_bass_kernel_spmd(
        nc, in_maps, core_ids=list(range(N_CORES)), trace=trace, **kwargs
    )
    parts = [res.results[c]["out"] for c in range(N_CORES)]
    out = np.stack(
        [
            parts[0] + parts[1] + parts[2] + parts[3],
            parts[4] + parts[5] + parts[6] + parts[7],
        ]
    ).astype(np.float32)
    return out, res


def kernel(**inputs):
    out, _ = _run(inputs, trace=False)
    return out


# revision 23
# speedup vs baseline: 1.0085x; 1.0085x over previous
"""Multi-head attention (B=2, S=2048, D=1024, H=16) on 8 TRN2 NeuronCores.

Sharding: DP over batch (2) x TP over heads (4 heads/core).
Core c: batch = c // 4, head group g = c % 4 -> heads [4g, 4g+4).

Per-core kernel (Tile):
  - activations arrive host-pre-transposed as [D, tokens] fp16 so the
    projection matmuls contract over the partition dim directly.
  - Q^T/K^T produced head-pair-stacked [128, 2048] fp16; V produced in
    natural [k, d] layout per head, augmented with a ones column (row 64
    of the PV output is then the softmax denominator for free).
  - S^T tiles [128 k, 2 x 512 q] in PSUM (2 heads row-packed on the PE),
    one ACT pass exp(S/8) -> fp16 (no max subtraction: logits ~ N(0,1)).
  - PV: O^T_h[65, q] accumulated over 16 k-blocks; evacuated with one DVE
    copy (frees PSUM fast), then normalized off the critical path via DVE
    reciprocal + GPSIMD partition-broadcast + DVE multiply.
  - o_proj: per-head 64-contraction matmuls accumulated in PSUM; partial
    [2048, 1024] outputs are summed on host across the 4 TP cores.
  - All projection / o_proj matmul groups are interleaved into the
    attention kb-loops as PE gap fillers so the PE never idles long
    enough for the HAM clock gate to re-throttle.
"""

import sys

if "/opt/trn_rl_repo" not in sys.path:
    sys.path.insert(0, "/opt/trn_rl_repo")

from contextlib import ExitStack

import numpy as np

import concourse.bacc as bacc
import concourse.bass as bass
import concourse.mybir as mybir
import concourse.tile as tile
from concourse.bass_utils import run_bass_kernel_spmd

F32 = mybir.dt.float32
F16 = mybir.dt.float16
EXP = mybir.ActivationFunctionType.Exp
ADD = mybir.AluOpType.add
MULT = mybir.AluOpType.mult

D = 1024  # model dim
TOK = 2048  # tokens per core (one batch element)
HL = 4  # heads per core
DH = 64  # head dim
TC = 512  # token/q chunk
NT = TOK // TC  # 4 q chunks
KB = TOK // 128  # 16 k blocks
NC8 = D // 128  # 8 contraction chunks
N_CORES = 8


def _build():
    nc = bacc.Bacc("TRN2", target_bir_lowering=False, debug=False, num_devices=N_CORES)

    qT_d = nc.declare_dram_parameter("qT", [D, TOK], F16, isOutput=False)
    kT_d = nc.declare_dram_parameter("kT", [D, TOK], F16, isOutput=False)
    vT_d = nc.declare_dram_parameter("vT", [D, TOK], F16, isOutput=False)
    wq_d = nc.declare_dram_parameter("wq", [D, 256], F16, isOutput=False)
    wk_d = nc.declare_dram_parameter("wk", [D, 256], F16, isOutput=False)
    wv_d = nc.declare_dram_parameter("wv", [D, 256], F16, isOutput=False)
    wo_d = nc.declare_dram_parameter("wo", [256, D], F16, isOutput=False)
    bq_d = nc.declare_dram_parameter("bq", [256, 1], F32, isOutput=False)
    bk_d = nc.declare_dram_parameter("bk", [256, 1], F32, isOutput=False)
    bv_d = nc.declare_dram_parameter("bv", [128, 256], F32, isOutput=False)
    bo_d = nc.declare_dram_parameter("bo", [128, D], F32, isOutput=False)
    out_d = nc.declare_dram_parameter("out", [TOK, D], F32, isOutput=True)

    with tile.TileContext(nc) as tc, ExitStack() as ctx:
        wts = ctx.enter_context(tc.tile_pool(name="wts", bufs=3))
        wop = ctx.enter_context(tc.tile_pool(name="wop", bufs=2))
        bp = ctx.enter_context(tc.tile_pool(name="bp", bufs=4))
        bvp = ctx.enter_context(tc.tile_pool(name="bvp", bufs=1))
        bop = ctx.enter_context(tc.tile_pool(name="bop", bufs=1))
        actp = ctx.enter_context(tc.tile_pool(name="actp", bufs=6))
        qkt = ctx.enter_context(tc.tile_pool(name="qkt", bufs=4))
        vga = ctx.enter_context(tc.tile_pool(name="vga", bufs=KB))
        p4p = ctx.enter_context(tc.tile_pool(name="p4p", bufs=6))
        otp = ctx.enter_context(tc.tile_pool(name="otp", bufs=4))
        oup = ctx.enter_context(tc.tile_pool(name="oup", bufs=3))
        rcp = ctx.enter_context(tc.tile_pool(name="rcp", bufs=2))
        bcp = ctx.enter_context(tc.tile_pool(name="bcp", bufs=2))
        ozp = ctx.enter_context(tc.tile_pool(name="ozp", bufs=6))
        pp = ctx.enter_context(tc.tile_pool(name="pp", bufs=2, space="PSUM"))
        ps = ctx.enter_context(tc.tile_pool(name="ps", bufs=2, space="PSUM"))
        po = ctx.enter_context(tc.tile_pool(name="po", bufs=1, space="PSUM"))

        def load_w(dram):
            # one 3D-AP DMA for all 8 contraction chunks: tile free dim is
            # (c8, col), DRAM partition dim is c8*128 + p.
            t_ = wts.tile([128, NC8 * 256], F16, tag="wts", name="wts_t")
            nc.sync.dma_start(
                t_[:].rearrange("p (c n) -> p c n", n=256),
                dram[:, :].rearrange("(c p) n -> p c n", p=128),
            )
            return [t_[:, c8 * 256 : (c8 + 1) * 256] for c8 in range(NC8)]

        def load_b(dram):
            lst = []
            for pair in range(2):
                t_ = bp.tile([128, 1], F32, tag="bp", name="b_t")
                nc.sync.dma_start(t_[:], dram[pair * 128 : (pair + 1) * 128, :])
                lst.append(t_)
            return lst

        KT = [qkt.tile([128, TOK], F16, tag="qkt", name=f"KT{i}") for i in range(2)]
        QT = [qkt.tile([128, TOK], F16, tag="qkt", name=f"QT{i}") for i in range(2)]

        def act_dma_set(src_d, t):
            ch = actp.tile([128, NC8 * TC], F16, tag="actp", name="act_ch")
            nc.sync.dma_start(
                ch[:].rearrange("p (c n) -> p c n", n=TC),
                src_d[:, t * TC : (t + 1) * TC].rearrange("(c p) n -> p c n", p=128),
            )
            return [ch[:, c8 * TC : (c8 + 1) * TC] for c8 in range(NC8)]

        def qk_half(dst, w_sb, b_sb, chunks, t, pair, half, box):
            if half == 0:
                box[1] = pp.tile([128, TC], F32, tag="pp", name="proj_ps")
            pt = box[1]
            for c8 in range(4 * half, 4 * half + 4):
                nc.tensor.matmul(
                    pt[:],
                    w_sb[c8][:, pair * 128 : (pair + 1) * 128],
                    chunks[c8],
                    start=(c8 == 0),
                    stop=(c8 == NC8 - 1),
                )
            if half == 1:
                nc.vector.tensor_scalar_add(
                    dst[pair][:, t * TC : (t + 1) * TC], pt[:], b_sb[pair][:]
                )

        def qk_pair(dst, w_sb, b_sb, chunks, t, pair):
            box = [None, None]
            qk_half(dst, w_sb, b_sb, chunks, t, pair, 0, box)
            qk_half(dst, w_sb, b_sb, chunks, t, pair, 1, box)

        # --- head of pipeline: K(0), Q(0), V chunks 0-1. Weight and
        # first-chunk DMAs are interleaved so the first matmul can start
        # after two transfers instead of eighteen. ---------------------
        # PE warm-up: junk matmuls during the DMA lead-in keep the HAM
        # activity monitor at full clock so real matmuls start warm.
        wu_in = wts.tile([128, TC], F16, tag="wu", name="wu_in")
        nc.vector.memset(wu_in[:], 0.0)
        wu_ps = pp.tile([128, TC], F32, tag="pp", name="wu_ps")
        for i in range(56):
            nc.tensor.matmul(
                wu_ps[:], wu_in[:, 0:128], wu_in[:], start=(i == 0), stop=(i == 55)
            )

        wk_sb = load_w(wk_d)
        kchunks0 = act_dma_set(kT_d, 0)
        bk_sb = load_b(bk_d)
        qk_pair(KT, wk_sb, bk_sb, kchunks0, 0, 0)
        qk_pair(KT, wk_sb, bk_sb, kchunks0, 0, 1)
        wq_sb = load_w(wq_d)
        qchunks0 = act_dma_set(qT_d, 0)
        bq_sb = load_b(bq_d)
        qk_pair(QT, wq_sb, bq_sb, qchunks0, 0, 0)
        qk_pair(QT, wq_sb, bq_sb, qchunks0, 0, 1)

        wv_sb = load_w(wv_d)
        bv_sb = bvp.tile([128, 256], F32, tag="bvp", name="bv_sb")
        nc.sync.dma_start(bv_sb[:], bv_d[:, :])
        wo_sb = []  # per head pair: [128, 1024]
        for hp in range(2):
            t_ = wop.tile([128, D], F16, tag="wop", name="wo_t")
            nc.sync.dma_start(t_[:], wo_d[hp * 128 : (hp + 1) * 128, :])
            wo_sb.append(t_)
        bo_sb = bop.tile([128, D], F32, tag="bop", name="bo_sb")
        nc.sync.dma_start(bo_sb[:], bo_d[:, :])

        vag = {}
        vchunk_sets = {}

        def v_dma_set(vt):
            vchunk_sets[vt] = act_dma_set(vT_d, vt)

        def v_group(kb):
            vt, j = divmod(kb, 4)
            vchunks = vchunk_sets[vt]
            pv = pp.tile([128, 256], F32, tag="pp", name="vproj_ps")
            for c8 in range(NC8):
                nc.tensor.matmul(
                    pv[:],
                    vchunks[c8][:, j * 128 : (j + 1) * 128],
                    wv_sb[c8],
                    start=(c8 == 0),
                    stop=(c8 == NC8 - 1),
                )
            # all 4 heads' V slices + bias in one strided DVE op; the ones
            # columns (one per head, stride 65) are set separately.
            vt_ = vga.tile([128, 4 * 65], F16, tag="vga", name="vag_t")
            dst = vt_[:].rearrange("p (h c) -> p h c", c=65)[:, :, 0:64]
            nc.vector.tensor_tensor(
                dst,
                pv[:].rearrange("p (h c) -> p h c", c=64),
                bv_sb[:].rearrange("p (h c) -> p h c", c=64),
                op=ADD,
            )
            nc.vector.memset(vt_[:].rearrange("p (h c) -> p h c", c=65)[:, :, 64:65], 1.0)
            for h in range(HL):
                vag[(h, kb)] = vt_[:, h * 65 : h * 65 + 65]

        v_dma_set(0)
        v_dma_set(1)
        v_dma_set(2)
        for kb in range(8):
            v_group(kb)

        ot_store = {}

        def oproj_group(t, qbl, f2):
            qb = 4 * t + qbl
            pz = pp.tile([128, TC], F32, tag="pp", name="oproj_ps")
            for hp in range(2):
                nc.tensor.matmul(
                    pz[:],
                    ot_store[(t, hp)][:, qbl * 128 : (qbl + 1) * 128],
                    wo_sb[hp][:, f2 * TC : (f2 + 1) * TC],
                    start=(hp == 0),
                    stop=(hp == 1),
                )
            oz = ozp.tile([128, TC], F32, tag="ozp", name="oz")
            nc.vector.tensor_tensor(
                oz[:], pz[:], bo_sb[:, f2 * TC : (f2 + 1) * TC], op=ADD
            )
            eng = nc.gpsimd if (qbl + f2) % 2 == 0 else nc.sync
            eng.dma_start(
                out_d[qb * 128 : (qb + 1) * 128, f2 * TC : (f2 + 1) * TC], oz[:]
            )

        def attn_pass(t, hp, fillers):
            """One head-pair kb-loop; fillers[kb] emits extra PE work.

            The S^T matmuls run two kb ahead of the PV matmuls (matching
            the two s4 PSUM slots) so a PV blocked on the accumulator WAR
            at pass start does not stall S^T/exp behind it on the
            in-order engines."""
            pO = po.tile([65, 1024], F32, tag="po", name="pO")
            p4s = {}

            def s_exp(kb):
                s4 = ps.tile([128, 1024], F32, tag="ps", name="s4")
                for par in range(2):
                    rows = slice(par * 64, par * 64 + 64)
                    nc.tensor.matmul(
                        s4[:, par * TC : (par + 1) * TC],
                        KT[hp][rows, kb * 128 : (kb + 1) * 128],
                        QT[hp][rows, t * TC : (t + 1) * TC],
                        start=True,
                        stop=True,
                    )
                p4 = p4p.tile([128, 1024], F16, tag="p4p", name="p4")
                nc.scalar.activation(p4[:], s4[:], EXP, scale=0.125)
                p4s[kb] = p4

            s_exp(0)
            for kb in range(KB):
                for f in fillers.get(kb, ()):
                    f()
                if kb + 1 < KB:
                    s_exp(kb + 1)
                p4 = p4s.pop(kb)
                for par in range(2):
                    h = 2 * hp + par
                    nc.tensor.matmul(
                        pO[:, par * TC : (par + 1) * TC],
                        vag[(h, kb)],
                        p4[:, par * TC : (par + 1) * TC],
                        start=(kb == 0),
                        stop=(kb == KB - 1),
                    )
            # fast evacuation frees the PSUM accumulator quickly; the
            # normalization chain runs off the PE critical path. The denom
            # row moves to partition 0 (partition_broadcast requires it).
            # On the very last pass nothing waits on the PSUM slot, so the
            # mults read PSUM directly and the copy is skipped.
            last = t == NT - 1 and hp == 1
            if not last:
                ou = oup.tile([64, 1024], F32, tag="oup", name="ou")
                nc.vector.tensor_copy(ou[:], pO[0:64, :])
            else:
                ou = pO
            rc = rcp.tile([1, 1024], F32, tag="rcp", name="rc")
            if last:
                nc.scalar.copy(rc[0:1, :], pO[64:65, :])
            else:
                nc.vector.tensor_copy(rc[0:1, :], pO[64:65, :])
            bd = bcp.tile([64, 1024], F32, tag="bcp", name="bd")
            nc.gpsimd.partition_broadcast(bd[:], rc[:], channels=64)
            bc = bcp.tile([64, 1024], F32, tag="bcp", name="bc")
            nc.vector.reciprocal_approx_fast(bc[:], bd[:])
            # pair-stacked normalized output: even head -> partitions 0:64,
            # odd head -> 64:128 (DVE partition-shifted write), so o_proj
            # contracts 128 rows per matmul.
            ot = otp.tile([128, TC], F16, tag="otp", name="ot")
            for par in range(2):
                nc.vector.tensor_tensor(
                    ot[par * 64 : (par + 1) * 64, :],
                    ou[0:64, par * TC : (par + 1) * TC],
                    bc[:, par * TC : (par + 1) * TC],
                    op=MULT,
                )
            ot_store[(t, hp)] = ot

        def kq_filler(src_d, dst, w_sb, b_sb, t, pair, half, box):
            def f():
                if box[0] is None:
                    box[0] = act_dma_set(src_d, t)
                qk_half(dst, w_sb, b_sb, box[0], t, pair, half, box)

            return f

        kbox = {t: [None, None] for t in (1, 2, 3)}
        qbox = {t: [None, None] for t in (1, 2, 3)}

        def kf(t, pair, half):
            return kq_filler(kT_d, KT, wk_sb, bk_sb, t, pair, half, kbox[t])

        def qf(t, pair, half):
            return kq_filler(qT_d, QT, wq_sb, bq_sb, t, pair, half, qbox[t])

        f00 = {
            0: [kf(1, 0, 0)],
            1: [kf(1, 0, 1)],
            2: [kf(1, 1, 0), lambda: v_dma_set(3)],
            3: [kf(1, 1, 1), lambda: v_group(8)],
            4: [kf(2, 0, 0), lambda: v_group(9)],
            5: [kf(2, 0, 1)],
            6: [kf(2, 1, 0), lambda: v_group(10)],
            7: [kf(2, 1, 1), lambda: v_group(11)],
            8: [lambda: v_group(12)],
            9: [lambda: v_group(13)],
            10: [kf(3, 0, 0), lambda: v_group(14)],
            11: [kf(3, 0, 1), lambda: v_group(15)],
            12: [kf(3, 1, 0)],
            13: [kf(3, 1, 1)],
        }
        # o_proj(t-1) depends on ot tiles that finish ~6us after pass
        # (t-1, hp1) ends (copy->bcast->recip->mult), so its groups go in
        # late hp0 slots / early hp1 slots of the next t. Q(t+1) projection
        # halves are spread across both passes for even PE load.
        for t in range(NT):
            if t == 0:
                f_hp0 = f00
                f_hp1 = {2 * j: [qf(1, j // 2, j % 2)] for j in range(4)}
            else:
                f_hp0 = {
                    7 + 2 * j: [lambda i=j, t=t: oproj_group(t - 1, i // 2, i % 2)]
                    for j in range(4)
                }
                f_hp1 = {}
                if t + 1 < NT:
                    for j in range(4):
                        f_hp0.setdefault(2 * j, []).append(qf(t + 1, j // 2, j % 2))
                for j in range(4):
                    f_hp1.setdefault(2 * j + 1, []).append(
                        lambda i=4 + j, t=t: oproj_group(t - 1, i // 2, i % 2)
                    )
            attn_pass(t, 0, f_hp0)
            attn_pass(t, 1, f_hp1)
        for i in range(8):
            oproj_group(NT - 1, i // 2, i % 2)

    nc.compile()
    return nc


_NC = None


def _get_nc():
    global _NC
    if _NC is None:
        _NC = _build()
    return _NC


def _shard(inputs):
    q = np.asarray(inputs["q"], np.float32)
    k = np.asarray(inputs["k"], np.float32)
    v = np.asarray(inputs["v"], np.float32)
    Wq = np.asarray(inputs["Wq"], np.float32)
    Wk = np.asarray(inputs["Wk"], np.float32)
    Wv = np.asarray(inputs["Wv"], np.float32)
    Wo = np.asarray(inputs["Wo"], np.float32)
    bq = np.asarray(inputs["bq"], np.float32)
    bk = np.asarray(inputs["bk"], np.float32)
    bv = np.asarray(inputs["bv"], np.float32)
    bo = np.asarray(inputs["bo"], np.float32)

    qT = [np.ascontiguousarray(q[b].T).astype(np.float16) for b in range(2)]
    kT = [np.ascontiguousarray(k[b].T).astype(np.float16) for b in range(2)]
    vT = [np.ascontiguousarray(v[b].T).astype(np.float16) for b in range(2)]
    bo_bc = np.tile((bo / 4.0).reshape(1, D), (128, 1)).astype(np.float32)

    in_maps = []
    for c in range(N_CORES):
        b, g = divmod(c, 4)
        sl = slice(g * 256, (g + 1) * 256)
        in_maps.append(
            {
                "qT": qT[b],
                "kT": kT[b],
                "vT": vT[b],
                "wq": np.ascontiguousarray(Wq[:, sl]).astype(np.float16),
                "wk": np.ascontiguousarray(Wk[:, sl]).astype(np.float16),
                "wv": np.ascontiguousarray(Wv[:, sl]).astype(np.float16),
                "wo": np.ascontiguousarray(Wo[sl, :]).astype(np.float16),
                "bq": np.ascontiguousarray(bq[sl].reshape(256, 1)),
                "bk": np.ascontiguousarray(bk[sl].reshape(256, 1)),
                "bv": np.tile(bv[sl].reshape(1, 256), (128, 1)).astype(np.float32),
                "bo": bo_bc,
            }
        )
    return in_maps


def _run(inputs, trace=False, **kwargs):
    nc = _get_nc()
    in_maps = _shard(inputs)
    res = run_bass_kernel_spmd(
        nc, in_maps, core_ids=list(range(N_CORES)), trace=trace, **kwargs
    )
    parts = [res.results[c]["out"] for c in range(N_CORES)]
    out = np.stack(
        [
            parts[0] + parts[1] + parts[2] + parts[3],
            parts[4] + parts[5] + parts[6] + parts[7],
        ]
    ).astype(np.float32)
    return out, res


def kernel(**inputs):
    out, _ = _run(inputs, trace=False)
    return out


# revision 24
# speedup vs baseline: 1.0182x; 1.0097x over previous
"""Multi-head attention (B=2, S=2048, D=1024, H=16) on 8 TRN2 NeuronCores.

Sharding: DP over batch (2) x TP over heads (4 heads/core).
Core c: batch = c // 4, head group g = c % 4 -> heads [4g, 4g+4).

Per-core kernel (Tile):
  - activations arrive host-pre-transposed as [D, tokens] fp16 so the
    projection matmuls contract over the partition dim directly.
  - Q^T/K^T produced head-pair-stacked [128, 2048] fp16; V produced in
    natural [k, d] layout per head, augmented with a ones column (row 64
    of the PV output is then the softmax denominator for free).
  - S^T tiles [128 k, 2 x 512 q] in PSUM (2 heads row-packed on the PE),
    one ACT pass exp(S/8) -> fp16 (no max subtraction: logits ~ N(0,1)).
  - PV: O^T_h[65, q] accumulated over 16 k-blocks; evacuated with one DVE
    copy (frees PSUM fast), then normalized off the critical path via DVE
    reciprocal + GPSIMD partition-broadcast + DVE multiply.
  - o_proj: per-head 64-contraction matmuls accumulated in PSUM; partial
    [2048, 1024] outputs are summed on host across the 4 TP cores.
  - All projection / o_proj matmul groups are interleaved into the
    attention kb-loops as PE gap fillers so the PE never idles long
    enough for the HAM clock gate to re-throttle.
"""

import sys

if "/opt/trn_rl_repo" not in sys.path:
    sys.path.insert(0, "/opt/trn_rl_repo")

from contextlib import ExitStack

import numpy as np

import concourse.bacc as bacc
import concourse.bass as bass
import concourse.mybir as mybir
import concourse.tile as tile
from concourse.bass_utils import run_bass_kernel_spmd

F32 = mybir.dt.float32
F16 = mybir.dt.float16
EXP = mybir.ActivationFunctionType.Exp
ADD = mybir.AluOpType.add
MULT = mybir.AluOpType.mult

D = 1024  # model dim
TOK = 2048  # tokens per core (one batch element)
HL = 4  # heads per core
DH = 64  # head dim
TC = 512  # token/q chunk
NT = TOK // TC  # 4 q chunks
KB = TOK // 128  # 16 k blocks
NC8 = D // 128  # 8 contraction chunks
N_CORES = 8


def _build():
    nc = bacc.Bacc("TRN2", target_bir_lowering=False, debug=False, num_devices=N_CORES)

    qT_d = nc.declare_dram_parameter("qT", [D, TOK], F16, isOutput=False)
    kT_d = nc.declare_dram_parameter("kT", [D, TOK], F16, isOutput=False)
    vT_d = nc.declare_dram_parameter("vT", [D, TOK], F16, isOutput=False)
    wq_d = nc.declare_dram_parameter("wq", [D, 256], F16, isOutput=False)
    wk_d = nc.declare_dram_parameter("wk", [D, 256], F16, isOutput=False)
    wv_d = nc.declare_dram_parameter("wv", [D, 256], F16, isOutput=False)
    wo_d = nc.declare_dram_parameter("wo", [256, D], F16, isOutput=False)
    bq_d = nc.declare_dram_parameter("bq", [256, 1], F32, isOutput=False)
    bk_d = nc.declare_dram_parameter("bk", [256, 1], F32, isOutput=False)
    bv_d = nc.declare_dram_parameter("bv", [128, 256], F32, isOutput=False)
    bo_d = nc.declare_dram_parameter("bo", [128, D], F32, isOutput=False)
    out_d = nc.declare_dram_parameter("out", [TOK, D], F32, isOutput=True)

    with tile.TileContext(nc) as tc, ExitStack() as ctx:
        wts = ctx.enter_context(tc.tile_pool(name="wts", bufs=3))
        wop = ctx.enter_context(tc.tile_pool(name="wop", bufs=2))
        bp = ctx.enter_context(tc.tile_pool(name="bp", bufs=4))
        bvp = ctx.enter_context(tc.tile_pool(name="bvp", bufs=1))
        bop = ctx.enter_context(tc.tile_pool(name="bop", bufs=1))
        actp = ctx.enter_context(tc.tile_pool(name="actp", bufs=6))
        qkt = ctx.enter_context(tc.tile_pool(name="qkt", bufs=4))
        vga = ctx.enter_context(tc.tile_pool(name="vga", bufs=KB))
        p4p = ctx.enter_context(tc.tile_pool(name="p4p", bufs=8))
        otp = ctx.enter_context(tc.tile_pool(name="otp", bufs=4))
        oup = ctx.enter_context(tc.tile_pool(name="oup", bufs=3))
        rcp = ctx.enter_context(tc.tile_pool(name="rcp", bufs=2))
        bcp = ctx.enter_context(tc.tile_pool(name="bcp", bufs=2))
        ozp = ctx.enter_context(tc.tile_pool(name="ozp", bufs=6))
        pp = ctx.enter_context(tc.tile_pool(name="pp", bufs=2, space="PSUM"))
        ps = ctx.enter_context(tc.tile_pool(name="ps", bufs=2, space="PSUM"))
        po = ctx.enter_context(tc.tile_pool(name="po", bufs=1, space="PSUM"))

        def load_w(dram):
            # one 3D-AP DMA for all 8 contraction chunks: tile free dim is
            # (c8, col), DRAM partition dim is c8*128 + p.
            t_ = wts.tile([128, NC8 * 256], F16, tag="wts", name="wts_t")
            nc.sync.dma_start(
                t_[:].rearrange("p (c n) -> p c n", n=256),
                dram[:, :].rearrange("(c p) n -> p c n", p=128),
            )
            return [t_[:, c8 * 256 : (c8 + 1) * 256] for c8 in range(NC8)]

        def load_b(dram):
            lst = []
            for pair in range(2):
                t_ = bp.tile([128, 1], F32, tag="bp", name="b_t")
                nc.sync.dma_start(t_[:], dram[pair * 128 : (pair + 1) * 128, :])
                lst.append(t_)
            return lst

        KT = [qkt.tile([128, TOK], F16, tag="qkt", name=f"KT{i}") for i in range(2)]
        QT = [qkt.tile([128, TOK], F16, tag="qkt", name=f"QT{i}") for i in range(2)]

        def act_dma_set(src_d, t):
            ch = actp.tile([128, NC8 * TC], F16, tag="actp", name="act_ch")
            nc.sync.dma_start(
                ch[:].rearrange("p (c n) -> p c n", n=TC),
                src_d[:, t * TC : (t + 1) * TC].rearrange("(c p) n -> p c n", p=128),
            )
            return [ch[:, c8 * TC : (c8 + 1) * TC] for c8 in range(NC8)]

        def qk_half(dst, w_sb, b_sb, chunks, t, pair, half, box):
            if half == 0:
                box[1] = pp.tile([128, TC], F32, tag="pp", name="proj_ps")
            pt = box[1]
            for c8 in range(4 * half, 4 * half + 4):
                nc.tensor.matmul(
                    pt[:],
                    w_sb[c8][:, pair * 128 : (pair + 1) * 128],
                    chunks[c8],
                    start=(c8 == 0),
                    stop=(c8 == NC8 - 1),
                )
            if half == 1:
                nc.vector.tensor_scalar_add(
                    dst[pair][:, t * TC : (t + 1) * TC], pt[:], b_sb[pair][:]
                )

        def qk_pair(dst, w_sb, b_sb, chunks, t, pair):
            box = [None, None]
            qk_half(dst, w_sb, b_sb, chunks, t, pair, 0, box)
            qk_half(dst, w_sb, b_sb, chunks, t, pair, 1, box)

        # --- head of pipeline: K(0), Q(0), V chunks 0-1. Weight and
        # first-chunk DMAs are interleaved so the first matmul can start
        # after two transfers instead of eighteen. ---------------------
        # PE warm-up: junk matmuls during the DMA lead-in keep the HAM
        # activity monitor at full clock so real matmuls start warm.
        wu_in = wts.tile([128, TC], F16, tag="wu", name="wu_in")
        nc.vector.memset(wu_in[:], 0.0)
        wu_ps = pp.tile([128, TC], F32, tag="pp", name="wu_ps")
        for i in range(56):
            nc.tensor.matmul(
                wu_ps[:], wu_in[:, 0:128], wu_in[:], start=(i == 0), stop=(i == 55)
            )

        wk_sb = load_w(wk_d)
        kchunks0 = act_dma_set(kT_d, 0)
        bk_sb = load_b(bk_d)
        qk_pair(KT, wk_sb, bk_sb, kchunks0, 0, 0)
        qk_pair(KT, wk_sb, bk_sb, kchunks0, 0, 1)
        wq_sb = load_w(wq_d)
        qchunks0 = act_dma_set(qT_d, 0)
        bq_sb = load_b(bq_d)
        qk_pair(QT, wq_sb, bq_sb, qchunks0, 0, 0)
        qk_pair(QT, wq_sb, bq_sb, qchunks0, 0, 1)

        wv_sb = load_w(wv_d)
        bv_sb = bvp.tile([128, 256], F32, tag="bvp", name="bv_sb")
        nc.sync.dma_start(bv_sb[:], bv_d[:, :])
        wo_sb = []  # per head pair: [128, 1024]
        for hp in range(2):
            t_ = wop.tile([128, D], F16, tag="wop", name="wo_t")
            nc.sync.dma_start(t_[:], wo_d[hp * 128 : (hp + 1) * 128, :])
            wo_sb.append(t_)
        bo_sb = bop.tile([128, D], F32, tag="bop", name="bo_sb")
        nc.sync.dma_start(bo_sb[:], bo_d[:, :])

        vag = {}
        vchunk_sets = {}

        def v_dma_set(vt):
            vchunk_sets[vt] = act_dma_set(vT_d, vt)

        def v_group(kb):
            vt, j = divmod(kb, 4)
            vchunks = vchunk_sets[vt]
            pv = pp.tile([128, 256], F32, tag="pp", name="vproj_ps")
            for c8 in range(NC8):
                nc.tensor.matmul(
                    pv[:],
                    vchunks[c8][:, j * 128 : (j + 1) * 128],
                    wv_sb[c8],
                    start=(c8 == 0),
                    stop=(c8 == NC8 - 1),
                )
            # all 4 heads' V slices + bias in one strided DVE op; the ones
            # columns (one per head, stride 65) are set separately.
            vt_ = vga.tile([128, 4 * 65], F16, tag="vga", name="vag_t")
            dst = vt_[:].rearrange("p (h c) -> p h c", c=65)[:, :, 0:64]
            nc.vector.tensor_tensor(
                dst,
                pv[:].rearrange("p (h c) -> p h c", c=64),
                bv_sb[:].rearrange("p (h c) -> p h c", c=64),
                op=ADD,
            )
            nc.vector.memset(vt_[:].rearrange("p (h c) -> p h c", c=65)[:, :, 64:65], 1.0)
            for h in range(HL):
                vag[(h, kb)] = vt_[:, h * 65 : h * 65 + 65]

        v_dma_set(0)
        v_dma_set(1)
        v_dma_set(2)
        for kb in range(8):
            v_group(kb)

        ot_store = {}

        def oproj_group(t, qbl, f2):
            qb = 4 * t + qbl
            pz = pp.tile([128, TC], F32, tag="pp", name="oproj_ps")
            for hp in range(2):
                nc.tensor.matmul(
                    pz[:],
                    ot_store[(t, hp)][:, qbl * 128 : (qbl + 1) * 128],
                    wo_sb[hp][:, f2 * TC : (f2 + 1) * TC],
                    start=(hp == 0),
                    stop=(hp == 1),
                )
            oz = ozp.tile([128, TC], F32, tag="ozp", name="oz")
            nc.vector.tensor_tensor(
                oz[:], pz[:], bo_sb[:, f2 * TC : (f2 + 1) * TC], op=ADD
            )
            eng = nc.gpsimd if (qbl + f2) % 2 == 0 else nc.sync
            eng.dma_start(
                out_d[qb * 128 : (qb + 1) * 128, f2 * TC : (f2 + 1) * TC], oz[:]
            )

        def attn_pass(t, hp, fillers):
            """One head-pair kb-loop; fillers[kb] emits extra PE work.

            The S^T matmuls run two kb ahead of the PV matmuls (matching
            the two s4 PSUM slots) so a PV blocked on the accumulator WAR
            at pass start does not stall S^T/exp behind it on the
            in-order engines."""
            pO = po.tile([65, 1024], F32, tag="po", name="pO")
            p4s = {}

            def s_exp(kb):
                s4 = ps.tile([128, 1024], F32, tag="ps", name="s4")
                for par in range(2):
                    rows = slice(par * 64, par * 64 + 64)
                    nc.tensor.matmul(
                        s4[:, par * TC : (par + 1) * TC],
                        KT[hp][rows, kb * 128 : (kb + 1) * 128],
                        QT[hp][rows, t * TC : (t + 1) * TC],
                        start=True,
                        stop=True,
                    )
                p4 = p4p.tile([128, 1024], F16, tag="p4p", name="p4")
                nc.scalar.activation(p4[:], s4[:], EXP, scale=0.125)
                p4s[kb] = p4

            s_exp(0)
            for kb in range(KB):
                for f in fillers.get(kb, ()):
                    f()
                if kb + 1 < KB:
                    s_exp(kb + 1)
                p4 = p4s.pop(kb)
                for par in range(2):
                    h = 2 * hp + par
                    nc.tensor.matmul(
                        pO[:, par * TC : (par + 1) * TC],
                        vag[(h, kb)],
                        p4[:, par * TC : (par + 1) * TC],
                        start=(kb == 0),
                        stop=(kb == KB - 1),
                    )
            # fast evacuation frees the PSUM accumulator quickly; the
            # normalization chain runs off the PE critical path. The denom
            # row moves to partition 0 (partition_broadcast requires it).
            # On the very last pass nothing waits on the PSUM slot, so the
            # mults read PSUM directly and the copy is skipped.
            last = t == NT - 1 and hp == 1
            if not last:
                # evacuate per PSUM bank (Tile tracks bank-level WAR deps,
                # so the next pass's first PV starts after the first half).
                ou = oup.tile([65, 1024], F32, tag="oup", name="ou")
                nc.vector.tensor_copy(ou[:, 0:TC], pO[:, 0:TC])
                nc.vector.tensor_copy(ou[:, TC:1024], pO[:, TC:1024])
                rc = rcp.tile([1, 1024], F32, tag="rcp", name="rc")
                nc.vector.tensor_copy(rc[0:1, :], ou[64:65, :])
            else:
                ou = pO
                rc = rcp.tile([1, 1024], F32, tag="rcp", name="rc")
                nc.scalar.copy(rc[0:1, :], pO[64:65, :])
            bd = bcp.tile([64, 1024], F32, tag="bcp", name="bd")
            nc.gpsimd.partition_broadcast(bd[:], rc[:], channels=64)
            bc = bcp.tile([64, 1024], F32, tag="bcp", name="bc")
            nc.vector.reciprocal_approx_fast(bc[:], bd[:])
            # pair-stacked normalized output: even head -> partitions 0:64,
            # odd head -> 64:128 (DVE partition-shifted write), so o_proj
            # contracts 128 rows per matmul.
            ot = otp.tile([128, TC], F16, tag="otp", name="ot")
            for par in range(2):
                nc.vector.tensor_tensor(
                    ot[par * 64 : (par + 1) * 64, :],
                    ou[0:64, par * TC : (par + 1) * TC],
                    bc[:, par * TC : (par + 1) * TC],
                    op=MULT,
                )
            ot_store[(t, hp)] = ot

        def kq_filler(src_d, dst, w_sb, b_sb, t, pair, half, box):
            def f():
                if box[0] is None:
                    box[0] = act_dma_set(src_d, t)
                qk_half(dst, w_sb, b_sb, box[0], t, pair, half, box)

            return f

        kbox = {t: [None, None] for t in (1, 2, 3)}
        qbox = {t: [None, None] for t in (1, 2, 3)}

        def kf(t, pair, half):
            return kq_filler(kT_d, KT, wk_sb, bk_sb, t, pair, half, kbox[t])

        def qf(t, pair, half):
            return kq_filler(qT_d, QT, wq_sb, bq_sb, t, pair, half, qbox[t])

        f00 = {
            0: [kf(1, 0, 0)],
            1: [kf(1, 0, 1)],
            2: [kf(1, 1, 0), lambda: v_dma_set(3)],
            3: [kf(1, 1, 1), lambda: v_group(8)],
            4: [kf(2, 0, 0), lambda: v_group(9)],
            5: [kf(2, 0, 1)],
            6: [kf(2, 1, 0), lambda: v_group(10)],
            7: [kf(2, 1, 1), lambda: v_group(11)],
            8: [lambda: v_group(12)],
            9: [lambda: v_group(13)],
            10: [kf(3, 0, 0), lambda: v_group(14)],
            11: [kf(3, 0, 1), lambda: v_group(15)],
            12: [kf(3, 1, 0)],
            13: [kf(3, 1, 1)],
        }
        # o_proj(t-1) depends on ot tiles that finish ~6us after pass
        # (t-1, hp1) ends (copy->bcast->recip->mult), so its groups go in
        # late hp0 slots / early hp1 slots of the next t. Q(t+1) projection
        # halves are spread across both passes for even PE load.
        for t in range(NT):
            if t == 0:
                f_hp0 = f00
                f_hp1 = {2 * j: [qf(1, j // 2, j % 2)] for j in range(4)}
            else:
                f_hp0 = {
                    7 + 2 * j: [lambda i=j, t=t: oproj_group(t - 1, i // 2, i % 2)]
                    for j in range(4)
                }
                f_hp1 = {}
                if t + 1 < NT:
                    for j in range(4):
                        f_hp0.setdefault(2 * j, []).append(qf(t + 1, j // 2, j % 2))
                for j in range(4):
                    f_hp1.setdefault(2 * j + 1, []).append(
                        lambda i=4 + j, t=t: oproj_group(t - 1, i // 2, i % 2)
                    )
            attn_pass(t, 0, f_hp0)
            attn_pass(t, 1, f_hp1)
        for i in range(8):
            oproj_group(NT - 1, i // 2, i % 2)

    nc.compile()
    return nc


_NC = None


def _get_nc():
    global _NC
    if _NC is None:
        _NC = _build()
    return _NC


def _shard(inputs):
    q = np.asarray(inputs["q"], np.float32)
    k = np.asarray(inputs["k"], np.float32)
    v = np.asarray(inputs["v"], np.float32)
    Wq = np.asarray(inputs["Wq"], np.float32)
    Wk = np.asarray(inputs["Wk"], np.float32)
    Wv = np.asarray(inputs["Wv"], np.float32)
    Wo = np.asarray(inputs["Wo"], np.float32)
    bq = np.asarray(inputs["bq"], np.float32)
    bk = np.asarray(inputs["bk"], np.float32)
    bv = np.asarray(inputs["bv"], np.float32)
    bo = np.asarray(inputs["bo"], np.float32)

    qT = [np.ascontiguousarray(q[b].T).astype(np.float16) for b in range(2)]
    kT = [np.ascontiguousarray(k[b].T).astype(np.float16) for b in range(2)]
    vT = [np.ascontiguousarray(v[b].T).astype(np.float16) for b in range(2)]
    bo_bc = np.tile((bo / 4.0).reshape(1, D), (128, 1)).astype(np.float32)

    in_maps = []
    for c in range(N_CORES):
        b, g = divmod(c, 4)
        sl = slice(g * 256, (g + 1) * 256)
        in_maps.append(
            {
                "qT": qT[b],
                "kT": kT[b],
                "vT": vT[b],
                "wq": np.ascontiguousarray(Wq[:, sl]).astype(np.float16),
                "wk": np.ascontiguousarray(Wk[:, sl]).astype(np.float16),
                "wv": np.ascontiguousarray(Wv[:, sl]).astype(np.float16),
                "wo": np.ascontiguousarray(Wo[sl, :]).astype(np.float16),
                "bq": np.ascontiguousarray(bq[sl].reshape(256, 1)),
                "bk": np.ascontiguousarray(bk[sl].reshape(256, 1)),
                "bv": np.tile(bv[sl].reshape(1, 256), (128, 1)).astype(np.float32),
                "bo": bo_bc,
            }
        )
    return in_maps


def _run(inputs, trace=False, **kwargs):
    nc = _get_nc()
    in_maps = _shard(inputs)
    res = run_bass_kernel_spmd(
        nc, in_maps, core_ids=list(range(N_CORES)), trace=trace, **kwargs
    )
    parts = [res.results[c]["out"] for c in range(N_CORES)]
    out = np.stack(
        [
            parts[0] + parts[1] + parts[2] + parts[3],
            parts[4] + parts[5] + parts[6] + parts[7],
        ]
    ).astype(np.float32)
    return out, res


def kernel(**inputs):
    out, _ = _run(inputs, trace=False)
    return out


# revision 25
# speedup vs baseline: 1.0269x; 1.0085x over previous
"""Multi-head attention (B=2, S=2048, D=1024, H=16) on 8 TRN2 NeuronCores.

Sharding: DP over batch (2) x TP over heads (4 heads/core).
Core c: batch = c // 4, head group g = c % 4 -> heads [4g, 4g+4).

Per-core kernel (Tile):
  - activations arrive host-pre-transposed as [D, tokens] fp16 so the
    projection matmuls contract over the partition dim directly.
  - Q^T/K^T produced head-pair-stacked [128, 2048] fp16; V produced in
    natural [k, d] layout per head, augmented with a ones column (row 64
    of the PV output is then the softmax denominator for free).
  - S^T tiles [128 k, 2 x 512 q] in PSUM (2 heads row-packed on the PE),
    one ACT pass exp(S/8) -> fp16 (no max subtraction: logits ~ N(0,1)).
  - PV: O^T_h[65, q] accumulated over 16 k-blocks; evacuated with one DVE
    copy (frees PSUM fast), then normalized off the critical path via DVE
    reciprocal + GPSIMD partition-broadcast + DVE multiply.
  - o_proj: per-head 64-contraction matmuls accumulated in PSUM; partial
    [2048, 1024] outputs are summed on host across the 4 TP cores.
  - All projection / o_proj matmul groups are interleaved into the
    attention kb-loops as PE gap fillers so the PE never idles long
    enough for the HAM clock gate to re-throttle.
"""

import sys

if "/opt/trn_rl_repo" not in sys.path:
    sys.path.insert(0, "/opt/trn_rl_repo")

from contextlib import ExitStack

import numpy as np

import concourse.bacc as bacc
import concourse.bass as bass
import concourse.mybir as mybir
import concourse.tile as tile
from concourse.bass_utils import run_bass_kernel_spmd

F32 = mybir.dt.float32
F16 = mybir.dt.float16
EXP = mybir.ActivationFunctionType.Exp
ADD = mybir.AluOpType.add
MULT = mybir.AluOpType.mult

D = 1024  # model dim
TOK = 2048  # tokens per core (one batch element)
HL = 4  # heads per core
DH = 64  # head dim
TC = 512  # token/q chunk
NT = TOK // TC  # 4 q chunks
KB = TOK // 128  # 16 k blocks
NC8 = D // 128  # 8 contraction chunks
N_CORES = 8


def _build():
    nc = bacc.Bacc("TRN2", target_bir_lowering=False, debug=False, num_devices=N_CORES)

    qT_d = nc.declare_dram_parameter("qT", [D, TOK], F16, isOutput=False)
    kT_d = nc.declare_dram_parameter("kT", [D, TOK], F16, isOutput=False)
    vT_d = nc.declare_dram_parameter("vT", [D, TOK], F16, isOutput=False)
    wq_d = nc.declare_dram_parameter("wq", [D, 256], F16, isOutput=False)
    wk_d = nc.declare_dram_parameter("wk", [D, 256], F16, isOutput=False)
    wv_d = nc.declare_dram_parameter("wv", [D, 256], F16, isOutput=False)
    wo_d = nc.declare_dram_parameter("wo", [256, D], F16, isOutput=False)
    bq_d = nc.declare_dram_parameter("bq", [256, 1], F32, isOutput=False)
    bk_d = nc.declare_dram_parameter("bk", [256, 1], F32, isOutput=False)
    bv_d = nc.declare_dram_parameter("bv", [128, 256], F32, isOutput=False)
    bo_d = nc.declare_dram_parameter("bo", [128, D], F32, isOutput=False)
    out_d = nc.declare_dram_parameter("out", [TOK, D], F32, isOutput=True)

    with tile.TileContext(nc) as tc, ExitStack() as ctx:
        wts = ctx.enter_context(tc.tile_pool(name="wts", bufs=3))
        wop = ctx.enter_context(tc.tile_pool(name="wop", bufs=2))
        bp = ctx.enter_context(tc.tile_pool(name="bp", bufs=4))
        bvp = ctx.enter_context(tc.tile_pool(name="bvp", bufs=1))
        bop = ctx.enter_context(tc.tile_pool(name="bop", bufs=1))
        actp = ctx.enter_context(tc.tile_pool(name="actp", bufs=6))
        qkt = ctx.enter_context(tc.tile_pool(name="qkt", bufs=4))
        vga = ctx.enter_context(tc.tile_pool(name="vga", bufs=KB))
        p4p = ctx.enter_context(tc.tile_pool(name="p4p", bufs=8))
        otp = ctx.enter_context(tc.tile_pool(name="otp", bufs=4))
        oup = ctx.enter_context(tc.tile_pool(name="oup", bufs=3))
        rcp = ctx.enter_context(tc.tile_pool(name="rcp", bufs=2))
        bcp = ctx.enter_context(tc.tile_pool(name="bcp", bufs=2))
        ozp = ctx.enter_context(tc.tile_pool(name="ozp", bufs=6))
        pp = ctx.enter_context(tc.tile_pool(name="pp", bufs=2, space="PSUM"))
        ps = ctx.enter_context(tc.tile_pool(name="ps", bufs=2, space="PSUM"))
        po = ctx.enter_context(tc.tile_pool(name="po", bufs=1, space="PSUM"))

        def load_w(dram):
            # one 3D-AP DMA for all 8 contraction chunks: tile free dim is
            # (c8, col), DRAM partition dim is c8*128 + p.
            t_ = wts.tile([128, NC8 * 256], F16, tag="wts", name="wts_t")
            nc.sync.dma_start(
                t_[:].rearrange("p (c n) -> p c n", n=256),
                dram[:, :].rearrange("(c p) n -> p c n", p=128),
            )
            return [t_[:, c8 * 256 : (c8 + 1) * 256] for c8 in range(NC8)]

        def load_b(dram):
            lst = []
            for pair in range(2):
                t_ = bp.tile([128, 1], F32, tag="bp", name="b_t")
                nc.sync.dma_start(t_[:], dram[pair * 128 : (pair + 1) * 128, :])
                lst.append(t_)
            return lst

        KT = [qkt.tile([128, TOK], F16, tag="qkt", name=f"KT{i}") for i in range(2)]
        QT = [qkt.tile([128, TOK], F16, tag="qkt", name=f"QT{i}") for i in range(2)]

        def act_dma_set(src_d, t):
            ch = actp.tile([128, NC8 * TC], F16, tag="actp", name="act_ch")
            nc.sync.dma_start(
                ch[:].rearrange("p (c n) -> p c n", n=TC),
                src_d[:, t * TC : (t + 1) * TC].rearrange("(c p) n -> p c n", p=128),
            )
            return [ch[:, c8 * TC : (c8 + 1) * TC] for c8 in range(NC8)]

        def qk_half(dst, w_sb, b_sb, chunks, t, pair, half, box):
            if half == 0:
                box[1] = pp.tile([128, TC], F32, tag="pp", name="proj_ps")
            pt = box[1]
            for c8 in range(4 * half, 4 * half + 4):
                nc.tensor.matmul(
                    pt[:],
                    w_sb[c8][:, pair * 128 : (pair + 1) * 128],
                    chunks[c8],
                    start=(c8 == 0),
                    stop=(c8 == NC8 - 1),
                )
            if half == 1:
                nc.vector.tensor_scalar_add(
                    dst[pair][:, t * TC : (t + 1) * TC], pt[:], b_sb[pair][:]
                )

        def qk_pair(dst, w_sb, b_sb, chunks, t, pair):
            box = [None, None]
            qk_half(dst, w_sb, b_sb, chunks, t, pair, 0, box)
            qk_half(dst, w_sb, b_sb, chunks, t, pair, 1, box)

        # --- head of pipeline: K(0), Q(0), V chunks 0-1. Weight and
        # first-chunk DMAs are interleaved so the first matmul can start
        # after two transfers instead of eighteen. ---------------------
        # PE warm-up: junk matmuls during the DMA lead-in keep the HAM
        # activity monitor at full clock so real matmuls start warm.
        wu_in = wts.tile([128, TC], F16, tag="wu", name="wu_in")
        nc.vector.memset(wu_in[:], 0.0)
        wu_ps = pp.tile([128, TC], F32, tag="pp", name="wu_ps")
        for i in range(56):
            nc.tensor.matmul(
                wu_ps[:], wu_in[:, 0:128], wu_in[:], start=(i == 0), stop=(i == 55)
            )

        wk_sb = load_w(wk_d)
        kchunks0 = act_dma_set(kT_d, 0)
        bk_sb = load_b(bk_d)
        qk_pair(KT, wk_sb, bk_sb, kchunks0, 0, 0)
        qk_pair(KT, wk_sb, bk_sb, kchunks0, 0, 1)
        wq_sb = load_w(wq_d)
        qchunks0 = act_dma_set(qT_d, 0)
        bq_sb = load_b(bq_d)
        qk_pair(QT, wq_sb, bq_sb, qchunks0, 0, 0)
        qk_pair(QT, wq_sb, bq_sb, qchunks0, 0, 1)

        wv_sb = load_w(wv_d)
        bv_sb = bvp.tile([128, 256], F32, tag="bvp", name="bv_sb")
        nc.sync.dma_start(bv_sb[:], bv_d[:, :])
        wo_sb = []  # per head pair: [128, 1024]
        for hp in range(2):
            t_ = wop.tile([128, D], F16, tag="wop", name="wo_t")
            nc.sync.dma_start(t_[:], wo_d[hp * 128 : (hp + 1) * 128, :])
            wo_sb.append(t_)
        bo_sb = bop.tile([128, D], F32, tag="bop", name="bo_sb")
        nc.sync.dma_start(bo_sb[:], bo_d[:, :])

        vag = {}
        vchunk_sets = {}

        def v_dma_set(vt):
            vchunk_sets[vt] = act_dma_set(vT_d, vt)

        def v_group(kb):
            vt, j = divmod(kb, 4)
            vchunks = vchunk_sets[vt]
            pv = pp.tile([128, 256], F32, tag="pp", name="vproj_ps")
            for c8 in range(NC8):
                nc.tensor.matmul(
                    pv[:],
                    vchunks[c8][:, j * 128 : (j + 1) * 128],
                    wv_sb[c8],
                    start=(c8 == 0),
                    stop=(c8 == NC8 - 1),
                )
            # all 4 heads' V slices + bias in one strided DVE op; the ones
            # columns (one per head, stride 65) are set separately.
            vt_ = vga.tile([128, 4 * 65], F16, tag="vga", name="vag_t")
            dst = vt_[:].rearrange("p (h c) -> p h c", c=65)[:, :, 0:64]
            nc.vector.tensor_tensor(
                dst,
                pv[:].rearrange("p (h c) -> p h c", c=64),
                bv_sb[:].rearrange("p (h c) -> p h c", c=64),
                op=ADD,
            )
            nc.vector.memset(vt_[:].rearrange("p (h c) -> p h c", c=65)[:, :, 64:65], 1.0)
            for h in range(HL):
                vag[(h, kb)] = vt_[:, h * 65 : h * 65 + 65]

        v_dma_set(0)
        v_dma_set(1)
        v_dma_set(2)
        for kb in range(8):
            v_group(kb)

        ot_store = {}

        def oproj_group(t, qbl, f2):
            qb = 4 * t + qbl
            pz = pp.tile([128, TC], F32, tag="pp", name="oproj_ps")
            for hp in range(2):
                nc.tensor.matmul(
                    pz[:],
                    ot_store[(t, hp)][:, qbl * 128 : (qbl + 1) * 128],
                    wo_sb[hp][:, f2 * TC : (f2 + 1) * TC],
                    start=(hp == 0),
                    stop=(hp == 1),
                )
            oz = ozp.tile([128, TC], F32, tag="ozp", name="oz")
            nc.vector.tensor_tensor(
                oz[:], pz[:], bo_sb[:, f2 * TC : (f2 + 1) * TC], op=ADD
            )
            eng = nc.gpsimd if (qbl + f2) % 2 == 0 else nc.sync
            eng.dma_start(
                out_d[qb * 128 : (qb + 1) * 128, f2 * TC : (f2 + 1) * TC], oz[:]
            )

        def attn_pass(t, hp, fillers):
            """One head-pair kb-loop; fillers[kb] emits extra PE work.

            The S^T matmuls run two kb ahead of the PV matmuls (matching
            the two s4 PSUM slots) so a PV blocked on the accumulator WAR
            at pass start does not stall S^T/exp behind it on the
            in-order engines."""
            pO = po.tile([65, 1024], F32, tag="po", name="pO")
            p4s = {}

            def s_exp(kb):
                s4 = ps.tile([128, 1024], F32, tag="ps", name="s4")
                for par in range(2):
                    rows = slice(par * 64, par * 64 + 64)
                    nc.tensor.matmul(
                        s4[:, par * TC : (par + 1) * TC],
                        KT[hp][rows, kb * 128 : (kb + 1) * 128],
                        QT[hp][rows, t * TC : (t + 1) * TC],
                        start=True,
                        stop=True,
                    )
                p4 = p4p.tile([128, 1024], F16, tag="p4p", name="p4")
                nc.scalar.activation(p4[:], s4[:], EXP, scale=0.125)
                p4s[kb] = p4

            s_exp(0)
            for kb in range(KB):
                for f in fillers.get(kb, ()):
                    f()
                if kb + 1 < KB:
                    s_exp(kb + 1)
                p4 = p4s.pop(kb)
                for par in range(2):
                    h = 2 * hp + par
                    nc.tensor.matmul(
                        pO[:, par * TC : (par + 1) * TC],
                        vag[(h, kb)],
                        p4[:, par * TC : (par + 1) * TC],
                        start=(kb == 0),
                        stop=(kb == KB - 1),
                    )
            # fast evacuation frees the PSUM accumulator quickly; the
            # normalization chain runs off the PE critical path. The denom
            # row moves to partition 0 (partition_broadcast requires it).
            # On the very last pass nothing waits on the PSUM slot, so the
            # mults read PSUM directly and the copy is skipped.
            last = t == NT - 1 and hp == 1
            if not last:
                # evacuate per PSUM bank (Tile tracks bank-level WAR deps,
                # so the next pass's first PV starts after the first half).
                ou = oup.tile([65, 1024], F32, tag="oup", name="ou")
                nc.vector.tensor_copy(ou[:, 0:TC], pO[:, 0:TC])
                nc.vector.tensor_copy(ou[:, TC:1024], pO[:, TC:1024])
                rc = rcp.tile([1, 1024], F32, tag="rcp", name="rc")
                nc.vector.tensor_copy(rc[0:1, :], ou[64:65, :])
            else:
                ou = pO
                rc = rcp.tile([1, 1024], F32, tag="rcp", name="rc")
                nc.scalar.copy(rc[0:1, :], pO[64:65, :])
            bd = bcp.tile([64, 1024], F32, tag="bcp", name="bd")
            nc.gpsimd.partition_broadcast(bd[:], rc[:], channels=64)
            bc = bcp.tile([64, 1024], F32, tag="bcp", name="bc")
            nc.vector.reciprocal_approx_fast(bc[:], bd[:])
            # pair-stacked normalized output: even head -> partitions 0:64,
            # odd head -> 64:128 (DVE partition-shifted write), so o_proj
            # contracts 128 rows per matmul.
            ot = otp.tile([128, TC], F16, tag="otp", name="ot")
            for par in range(2):
                nc.vector.tensor_tensor(
                    ot[par * 64 : (par + 1) * 64, :],
                    ou[0:64, par * TC : (par + 1) * TC],
                    bc[:, par * TC : (par + 1) * TC],
                    op=MULT,
                )
            ot_store[(t, hp)] = ot

        def kq_filler(src_d, dst, w_sb, b_sb, t, pair, half, box):
            def f():
                if box[0] is None:
                    box[0] = act_dma_set(src_d, t)
                qk_half(dst, w_sb, b_sb, box[0], t, pair, half, box)

            return f

        kbox = {t: [None, None] for t in (1, 2, 3)}
        qbox = {t: [None, None] for t in (1, 2, 3)}

        def kf(t, pair, half):
            return kq_filler(kT_d, KT, wk_sb, bk_sb, t, pair, half, kbox[t])

        def qf(t, pair, half):
            return kq_filler(qT_d, QT, wq_sb, bq_sb, t, pair, half, qbox[t])

        f00 = {
            0: [kf(1, 0, 0)],
            1: [kf(1, 0, 1)],
            2: [kf(1, 1, 0), lambda: v_dma_set(3)],
            3: [kf(1, 1, 1), lambda: v_group(8)],
            4: [kf(2, 0, 0), lambda: v_group(9)],
            5: [kf(2, 0, 1)],
            6: [kf(2, 1, 0), lambda: v_group(10)],
            7: [kf(2, 1, 1), lambda: v_group(11)],
            8: [lambda: v_group(12)],
            9: [lambda: v_group(13)],
            10: [kf(3, 0, 0), lambda: v_group(14)],
            11: [kf(3, 0, 1), lambda: v_group(15)],
            12: [kf(3, 1, 0)],
            13: [kf(3, 1, 1)],
        }
        # o_proj(t-1) depends on ot tiles that finish ~6us after pass
        # (t-1, hp1) ends (copy->bcast->recip->mult), so its groups go in
        # late hp0 slots / early hp1 slots of the next t. Q(t+1) projection
        # halves are spread across both passes for even PE load.
        for t in range(NT):
            if t == 0:
                f_hp0 = f00
                f_hp1 = {2 * j: [qf(1, j // 2, j % 2)] for j in range(4)}
            else:
                f_hp0 = {
                    7 + 2 * j: [lambda i=j, t=t: oproj_group(t - 1, i // 2, i % 2)]
                    for j in range(4)
                }
                f_hp1 = {}
                if t + 1 < NT:
                    for j in range(4):
                        f_hp0.setdefault(2 * j, []).append(qf(t + 1, j // 2, j % 2))
                for j in range(4):
                    f_hp1.setdefault(2 * j + 1, []).append(
                        lambda i=4 + j, t=t: oproj_group(t - 1, i // 2, i % 2)
                    )
            attn_pass(t, 0, f_hp0)
            attn_pass(t, 1, f_hp1)
        for i in range(8):
            oproj_group(NT - 1, i // 2, i % 2)

    nc.compile()
    return nc


_NC = None


def _get_nc():
    global _NC
    if _NC is None:
        _NC = _build()
    return _NC


def _shard(inputs):
    q = np.asarray(inputs["q"], np.float32)
    k = np.asarray(inputs["k"], np.float32)
    v = np.asarray(inputs["v"], np.float32)
    Wq = np.asarray(inputs["Wq"], np.float32)
    Wk = np.asarray(inputs["Wk"], np.float32)
    Wv = np.asarray(inputs["Wv"], np.float32)
    Wo = np.asarray(inputs["Wo"], np.float32)
    bq = np.asarray(inputs["bq"], np.float32)
    bk = np.asarray(inputs["bk"], np.float32)
    bv = np.asarray(inputs["bv"], np.float32)
    bo = np.asarray(inputs["bo"], np.float32)

    qT = [np.ascontiguousarray(q[b].T).astype(np.float16) for b in range(2)]
    kT = [np.ascontiguousarray(k[b].T).astype(np.float16) for b in range(2)]
    vT = [np.ascontiguousarray(v[b].T).astype(np.float16) for b in range(2)]
    bo_bc = np.tile((bo / 4.0).reshape(1, D), (128, 1)).astype(np.float32)

    in_maps = []
    for c in range(N_CORES):
        b, g = divmod(c, 4)
        sl = slice(g * 256, (g + 1) * 256)
        in_maps.append(
            {
                "qT": qT[b],
                "kT": kT[b],
                "vT": vT[b],
                "wq": np.ascontiguousarray(Wq[:, sl]).astype(np.float16),
                "wk": np.ascontiguousarray(Wk[:, sl]).astype(np.float16),
                "wv": np.ascontiguousarray(Wv[:, sl]).astype(np.float16),
                "wo": np.ascontiguousarray(Wo[sl, :]).astype(np.float16),
                "bq": np.ascontiguousarray(bq[sl].reshape(256, 1)),
                "bk": np.ascontiguousarray(bk[sl].reshape(256, 1)),
                "bv": np.tile(bv[sl].reshape(1, 256), (128, 1)).astype(np.float32),
                "bo": bo_bc,
            }
        )
    return in_maps


def _run(inputs, trace=False, **kwargs):
    nc = _get_nc()
    in_maps = _shard(inputs)
    res = None
    for attempt in range(3):
        try:
            res = run_bass_kernel_spmd(
                nc, in_maps, core_ids=list(range(N_CORES)), trace=trace, **kwargs
            )
            break
        except Exception:
            if attempt == 2:
                raise
    parts = [res.results[c]["out"] for c in range(N_CORES)]
    out = np.stack(
        [
            parts[0] + parts[1] + parts[2] + parts[3],
            parts[4] + parts[5] + parts[6] + parts[7],
        ]
    ).astype(np.float32)
    return out, res


def kernel(**inputs):
    out, _ = _run(inputs, trace=False)
    return out
